# revision 6
# baseline (speedup 1.0000x reference)
"""Multi-head causal attention (B=2, T=2048, C=1024, H=16, HS=64) on 8 TRN2
NeuronCores.

Sharding: 2 heads per core (tensor parallel). Each core receives the full
(pre-transposed) activations xT [B, C, T], its 2 heads' QKV weight slices
packed [C, 128], and its 128-column slice of w_proj transposed [128, C].
Each core computes a partial output [B, T, C]; the host sums the 8 partials
and adds b_proj.

Per-core kernel (all matmuls in float32r — tf32-like, 1 cycle/row):
  - QT/KT/VT [128(2 heads x 64), T] via lhsT=weight chunks, rhs=xT chunks.
  - V_aug [keys, 128]: V (cols 0:64, via PE-transpose of VT) | ones (64:128).
  - Flash-style causal attention in transposed layout: S^T[keys, q] blocks
    via lhsT=KT block, rhs=QT slice; exp on ScalarE (no max subtraction --
    scores are O(1) by construction); O^T = [V|1].T @ P^T accumulated over
    key blocks gives both O rows (0:64) and the softmax sums l replicated
    (rows 64:128) in one pass.
  - Normalize with reciprocal_approx_fast + mixed-base tensor_tensor.
  - Output projection: lhsT = OhatT t-chunks, rhs = w_projT slice.
"""

import sys

if "/opt/trn_rl_repo" not in sys.path:
    sys.path.insert(0, "/opt/trn_rl_repo")

import math
from contextlib import ExitStack

import numpy as np

import concourse.mybir as mybir
import concourse.tile as tile
from concourse import bacc
from concourse.bass import ts
from concourse.bass_utils import run_bass_kernel_spmd

B, T, C = 2, 2048, 1024
H, HS = 16, 64
NCORES = 8
HPC = H // NCORES  # 2 heads per core
P = 128
G = 512  # q-group size
NG = T // G  # 4 q-groups
KB = 128  # key block
NPO = C // P  # 8 contraction chunks
F32 = mybir.dt.float32
F32R = mybir.dt.float32r

_nc_cache = {}


def _emit(tc):
    nc = tc.nc
    xt = nc.dram_tensor("xt", [B, C, T], F32R, kind="ExternalInput").ap()
    wq2 = nc.dram_tensor("wq2", [C, 128], F32R, kind="ExternalInput").ap()
    wk2 = nc.dram_tensor("wk2", [C, 128], F32R, kind="ExternalInput").ap()
    wv2 = nc.dram_tensor("wv2", [C, 128], F32R, kind="ExternalInput").ap()
    wpt = nc.dram_tensor("wpt", [128, C], F32R, kind="ExternalInput").ap()
    tri = nc.dram_tensor("tri", [P, P], F32R, kind="ExternalInput").ap()
    out = nc.dram_tensor("out", [B, T, C], F32, kind="ExternalOutput").ap()

    ctx = ExitStack()
    persist = ctx.enter_context(tc.tile_pool(name="persist", bufs=1))
    xt_pool = ctx.enter_context(tc.tile_pool(name="xtp", bufs=1))
    qk_pool = ctx.enter_context(tc.tile_pool(name="qkp", bufs=2))
    vt_pool = ctx.enter_context(tc.tile_pool(name="vtp", bufs=1))
    vaug_pool = ctx.enter_context(tc.tile_pool(name="vaugp", bufs=2))
    pt_pool = ctx.enter_context(tc.tile_pool(name="ptp", bufs=3))
    norm_pool = ctx.enter_context(tc.tile_pool(name="normp", bufs=2))
    ohat_pool = ctx.enter_context(tc.tile_pool(name="ohatp", bufs=2))
    out_pool = ctx.enter_context(tc.tile_pool(name="outp", bufs=2))
    mm_psum = ctx.enter_context(tc.tile_pool(name="mmps", bufs=3, space="PSUM"))
    ot_psum = ctx.enter_context(tc.tile_pool(name="otps", bufs=2, space="PSUM"))

    # persistent weights / masks
    wq_sb = persist.tile([P, NPO, 128], F32R, tag="wq")
    wk_sb = persist.tile([P, NPO, 128], F32R, tag="wk")
    wv_sb = persist.tile([P, NPO, 128], F32R, tag="wv")
    nc.sync.dma_start(wq_sb[:], wq2.rearrange("(po pi) d -> pi po d", pi=P))
    nc.sync.dma_start(wk_sb[:], wk2.rearrange("(po pi) d -> pi po d", pi=P))
    nc.sync.dma_start(wv_sb[:], wv2.rearrange("(po pi) d -> pi po d", pi=P))
    wpt_sb = persist.tile([P, C], F32R, tag="wpt")
    nc.sync.dma_start(wpt_sb[:], wpt[:])
    tri_sb = persist.tile([P, P], F32R, tag="tri")
    nc.sync.dma_start(tri_sb[:], tri[:])
    # [128, 64] identity (both partition halves = I64) for VT transposes
    identd = nc.dram_tensor("ident", [P, 64], F32R, kind="ExternalInput").ap()
    ident = persist.tile([P, 64], F32R, tag="ident")
    nc.sync.dma_start(ident[:], identd[:])
    onesd = nc.dram_tensor("ones", [P, T // KB, 64], F32R, kind="ExternalInput").ap()

    for b in range(B):
        # ---- load xT[b] ----
        xt_sb = xt_pool.tile([P, NPO, T], F32R, tag="xt")
        for po in range(NPO):
            nc.sync.dma_start(xt_sb[:, po, :], xt[b, po * P : (po + 1) * P, :])

        # ---- QT / KT / VT projections: [128(2h x 64), T] ----
        qt = qk_pool.tile([P, T], F32R, tag="qt")
        kt = qk_pool.tile([P, T], F32R, tag="kt")
        vt = vt_pool.tile([P, T], F32R, tag="vt")
        for w_sb, dst in ((wq_sb, qt), (wk_sb, kt), (wv_sb, vt)):
            for tg in range(T // 512):
                ps = mm_psum.tile([P, 512], F32, tag="mm")
                for po in range(NPO):
                    nc.tensor.matmul(
                        ps[:],
                        w_sb[:, po, :],
                        xt_sb[:, po, ts(tg, 512)],
                        start=(po == 0),
                        stop=(po == NPO - 1),
                    )
                nc.vector.tensor_copy(dst[:, ts(tg, 512)], ps[:])

        # ---- V_aug per head: [keys(P), 16 kb, 128] = [V | ones] ----
        vaug = []
        for h in range(HPC):
            va = vaug_pool.tile([P, T // KB, 128], F32R, tag=f"vaug{h}")
            nc.sync.dma_start(va[:, :, 64:128], onesd[:])
            for kbg in range(4):
                tp = mm_psum.tile([P, 4, 64], F32R, tag="mm")
                for kk in range(4):
                    kb = 4 * kbg + kk
                    nc.tensor.transpose(
                        tp[:, kk, :],
                        vt[64 * h : 64 * h + 64, ts(kb, KB)],
                        ident[64 * h : 64 * h + 64, :],
                    )
                nc.vector.tensor_copy(va[:, 4 * kbg : 4 * kbg + 4, 0:64], tp[:])
            vaug.append(va)

        # ---- attention + projection, per q-group ----
        ohat = ohat_pool.tile([P, T], F32R, tag="ohat")
        for g in range(NG):
            l_sb = norm_pool.tile([P, G], F32, tag="lsb")
            rinv = norm_pool.tile([P, G], F32, tag="rinv")
            otps_h = []
            for h in range(HPC):
                hb = 64 * h
                otps = ot_psum.tile([P, G], F32, tag="ot")
                n_j = 4 * g + 4
                for jg in range(math.ceil(n_j / 2)):
                    js = [j for j in (2 * jg, 2 * jg + 1) if j < n_j]
                    stps = mm_psum.tile([P, 2, G], F32, tag="mm")
                    pt = pt_pool.tile([P, 2, G], F32R, tag="pt")
                    diag = 2 * jg >= 4 * g
                    for idx, j in enumerate(js):
                        r = j - 4 * g
                        q0 = 128 * r if r >= 0 else 0
                        nc.tensor.matmul(
                            stps[:, idx, q0:G],
                            kt[hb : hb + 64, ts(j, KB)],
                            qt[hb : hb + 64, G * g + q0 : G * (g + 1)],
                            start=True,
                            stop=True,
                        )
                    if not diag:
                        nc.scalar.activation(
                            pt[:, :, :],
                            stps[:, :, :],
                            mybir.ActivationFunctionType.Exp,
                            scale=float(HS) ** -0.5,
                        )
                    else:
                        for idx, j in enumerate(js):
                            q0 = 128 * (j - 4 * g)
                            nc.scalar.activation(
                                pt[:, idx, q0:G],
                                stps[:, idx, q0:G],
                                mybir.ActivationFunctionType.Exp,
                                scale=float(HS) ** -0.5,
                            )
                            nc.vector.tensor_tensor(
                                pt[:, idx, q0 : q0 + 128],
                                pt[:, idx, q0 : q0 + 128],
                                tri_sb[:],
                                mybir.AluOpType.mult,
                            )
                    for idx, j in enumerate(js):
                        r = j - 4 * g
                        q0 = 128 * r if r >= 0 else 0
                        nc.tensor.matmul(
                            otps[:, q0:G],
                            vaug[h][:, j, :],
                            pt[:, idx, q0:G],
                            start=(j == 0),
                            stop=(j == n_j - 1),
                        )
                # softmax sums (rows 64:128, replicated) -> l_sb rows 64h
                nc.vector.tensor_copy(l_sb[hb : hb + 64, :], otps[64:128, :])
                otps_h.append(otps)
            nc.vector.reciprocal_approx_fast(rinv[:], l_sb[:])
            for h in range(HPC):
                hb = 64 * h
                nc.vector.tensor_tensor(
                    ohat[hb : hb + 64, ts(g, G)],
                    otps_h[h][0:64, :],
                    rinv[hb : hb + 64, :],
                    mybir.AluOpType.mult,
                )
            # ---- output projection for this q-group ----
            for tc4 in range(G // P):
                t0 = G * g + P * tc4
                pj = mm_psum.tile([P, C], F32, tag="mm")
                for n in range(C // 512):
                    nc.tensor.matmul(
                        pj[:, ts(n, 512)],
                        ohat[:, t0 : t0 + P],
                        wpt_sb[:, ts(n, 512)],
                        start=True,
                        stop=True,
                    )
                o_sb = out_pool.tile([P, C], F32, tag="osb")
                if tc4 % 2 == 0:
                    nc.vector.tensor_copy(o_sb[:], pj[:])
                else:
                    nc.scalar.copy(o_sb[:], pj[:])
                nc.sync.dma_start(out[b, t0 : t0 + P, :], o_sb[:])
    ctx.close()


def _build():
    if "nc" in _nc_cache:
        return _nc_cache["nc"]
    nc = bacc.Bacc("TRN2", target_bir_lowering=False, debug=False)
    with tile.TileContext(nc) as tc:
        _emit(tc)
    nc.compile()
    _nc_cache["nc"] = nc
    return nc


def kernel(x, wq, wk, wv, w_proj, b_proj):
    x = np.asarray(x, dtype=np.float32)
    wq = np.asarray(wq, dtype=np.float32)
    wk = np.asarray(wk, dtype=np.float32)
    wv = np.asarray(wv, dtype=np.float32)
    w_proj = np.asarray(w_proj, dtype=np.float32)
    b_proj = np.asarray(b_proj, dtype=np.float32)

    nc = _build()

    xt = np.ascontiguousarray(x.transpose(0, 2, 1))  # [B, C, T]
    tri = np.triu(np.ones((P, P), dtype=np.float32))  # keep key <= query
    ident = np.tile(np.eye(64, dtype=np.float32), (2, 1))  # [128, 64]
    ones = np.ones((P, T // KB, 64), dtype=np.float32)

    in_maps = []
    for c in range(NCORES):
        h0 = HPC * c
        in_maps.append(
            {
                "xt": xt,
                "wq2": np.ascontiguousarray(
                    np.concatenate([wq[h0 + i] for i in range(HPC)], axis=1)
                ),
                "wk2": np.ascontiguousarray(
                    np.concatenate([wk[h0 + i] for i in range(HPC)], axis=1)
                ),
                "wv2": np.ascontiguousarray(
                    np.concatenate([wv[h0 + i] for i in range(HPC)], axis=1)
                ),
                "wpt": np.ascontiguousarray(
                    w_proj[:, 128 * c : 128 * (c + 1)].T
                ),
                "tri": tri,
                "ident": ident,
                "ones": ones,
            }
        )

    res = run_bass_kernel_spmd(nc, in_maps, core_ids=list(range(NCORES)))
    acc = np.zeros((B, T, C), dtype=np.float64)
    for r in res.results:
        acc += r["out"]
    return (acc + b_proj).astype(np.float32)


# revision 8
# speedup vs baseline: 1.0251x; 1.0251x over previous
"""Multi-head causal attention (B=2, T=2048, C=1024, H=16, HS=64) on 8 TRN2
NeuronCores.

Sharding: 2 heads per core (tensor parallel). Each core receives the full
(pre-transposed) activations xT [B, C, T], its 2 heads' QKV weight slices
packed [C, 128], and its 128-column slice of w_proj transposed [128, C].
Each core computes a partial output [B, T, C]; the host sums the 8 partials
and adds b_proj.

Per-core kernel (all matmuls in float32r — tf32-like, 1 cycle/row):
  - QT/KT/VT [128(2 heads x 64), T] via lhsT=weight chunks, rhs=xT chunks.
  - V_aug [keys, 128]: V (cols 0:64, via PE-transpose of VT) | ones (64:128).
  - Flash-style causal attention in transposed layout: S^T[keys, q] blocks
    via lhsT=KT block, rhs=QT slice; exp on ScalarE (no max subtraction --
    scores are O(1) by construction); O^T = [V|1].T @ P^T accumulated over
    key blocks gives both O rows (0:64) and the softmax sums l replicated
    (rows 64:128) in one pass.
  - Normalize with reciprocal_approx_fast + mixed-base tensor_tensor.
  - Output projection: lhsT = OhatT t-chunks, rhs = w_projT slice.
"""

import sys

if "/opt/trn_rl_repo" not in sys.path:
    sys.path.insert(0, "/opt/trn_rl_repo")

import math
from contextlib import ExitStack

import numpy as np

import concourse.mybir as mybir
import concourse.tile as tile
from concourse import bacc
from concourse.bass import ts
from concourse.bass_utils import run_bass_kernel_spmd

B, T, C = 2, 2048, 1024
H, HS = 16, 64
NCORES = 8
HPC = H // NCORES  # 2 heads per core
P = 128
G = 512  # q-group size
NG = T // G  # 4 q-groups
KB = 128  # key block
NPO = C // P  # 8 contraction chunks
F32 = mybir.dt.float32
F32R = mybir.dt.float32r

_nc_cache = {}


def _emit(tc):
    nc = tc.nc
    xt = nc.dram_tensor("xt", [B, C, T], F32R, kind="ExternalInput").ap()
    wq2 = nc.dram_tensor("wq2", [C, 128], F32R, kind="ExternalInput").ap()
    wk2 = nc.dram_tensor("wk2", [C, 128], F32R, kind="ExternalInput").ap()
    wv2 = nc.dram_tensor("wv2", [C, 128], F32R, kind="ExternalInput").ap()
    wpt = nc.dram_tensor("wpt", [128, C], F32R, kind="ExternalInput").ap()
    tri = nc.dram_tensor("tri", [P, P], F32R, kind="ExternalInput").ap()
    out = nc.dram_tensor("out", [B, T, C], F32, kind="ExternalOutput").ap()

    ctx = ExitStack()
    persist = ctx.enter_context(tc.tile_pool(name="persist", bufs=1))
    xt_pool = ctx.enter_context(tc.tile_pool(name="xtp", bufs=1))
    qk_pool = ctx.enter_context(tc.tile_pool(name="qkp", bufs=2))
    vt_pool = ctx.enter_context(tc.tile_pool(name="vtp", bufs=1))
    vaug_pool = ctx.enter_context(tc.tile_pool(name="vaugp", bufs=2))
    pt_pool = ctx.enter_context(tc.tile_pool(name="ptp", bufs=3))
    norm_pool = ctx.enter_context(tc.tile_pool(name="normp", bufs=2))
    ohat_pool = ctx.enter_context(tc.tile_pool(name="ohatp", bufs=2))
    out_pool = ctx.enter_context(tc.tile_pool(name="outp", bufs=2))
    mm_psum = ctx.enter_context(tc.tile_pool(name="mmps", bufs=3, space="PSUM"))
    ot_psum = ctx.enter_context(tc.tile_pool(name="otps", bufs=2, space="PSUM"))

    # persistent weights / masks
    wq_sb = persist.tile([P, NPO, 128], F32R, tag="wq")
    wk_sb = persist.tile([P, NPO, 128], F32R, tag="wk")
    wv_sb = persist.tile([P, NPO, 128], F32R, tag="wv")
    nc.sync.dma_start(wq_sb[:], wq2.rearrange("(po pi) d -> pi po d", pi=P))
    nc.sync.dma_start(wk_sb[:], wk2.rearrange("(po pi) d -> pi po d", pi=P))
    nc.sync.dma_start(wv_sb[:], wv2.rearrange("(po pi) d -> pi po d", pi=P))
    wpt_sb = persist.tile([P, C], F32R, tag="wpt")
    nc.sync.dma_start(wpt_sb[:], wpt[:])
    tri_sb = persist.tile([P, P], F32R, tag="tri")
    nc.sync.dma_start(tri_sb[:], tri[:])
    # [128, 64] identity (both partition halves = I64) for VT transposes
    identd = nc.dram_tensor("ident", [P, 64], F32R, kind="ExternalInput").ap()
    ident = persist.tile([P, 64], F32R, tag="ident")
    nc.sync.dma_start(ident[:], identd[:])
    onesd = nc.dram_tensor("ones", [P, T // KB, 64], F32R, kind="ExternalInput").ap()

    for b in range(B):
        # ---- load xT[b] ----
        xt_sb = xt_pool.tile([P, NPO, T], F32R, tag="xt")
        for po in range(NPO):
            nc.sync.dma_start(xt_sb[:, po, :], xt[b, po * P : (po + 1) * P, :])

        # ---- QT / KT / VT projections: [128(2h x 64), T] ----
        qt = qk_pool.tile([P, T], F32R, tag="qt")
        kt = qk_pool.tile([P, T], F32R, tag="kt")
        vt = vt_pool.tile([P, T], F32R, tag="vt")
        for w_sb, dst in ((wq_sb, qt), (wk_sb, kt), (wv_sb, vt)):
            for tg in range(T // 512):
                ps = mm_psum.tile([P, 512], F32, tag="mm")
                for po in range(NPO):
                    nc.tensor.matmul(
                        ps[:],
                        w_sb[:, po, :],
                        xt_sb[:, po, ts(tg, 512)],
                        start=(po == 0),
                        stop=(po == NPO - 1),
                    )
                nc.vector.tensor_copy(dst[:, ts(tg, 512)], ps[:])

        # ---- V_aug per head: [keys(P), 16 kb, 128] = [V | ones] ----
        vaug = []
        for h in range(HPC):
            va = vaug_pool.tile([P, T // KB, 128], F32R, tag=f"vaug{h}")
            nc.sync.dma_start(va[:, :, 64:128], onesd[:])
            vaug.append(va)
        for kbg in range(4):
            tps = [mm_psum.tile([P, 4, 64], F32R, tag="mm", name=f"vtr{h}") for h in range(HPC)]
            for kk in range(4):
                kb = 4 * kbg + kk
                for h in range(HPC):
                    nc.tensor.transpose(
                        tps[h][:, kk, :],
                        vt[64 * h : 64 * h + 64, ts(kb, KB)],
                        ident[64 * h : 64 * h + 64, :],
                    )
            for h in range(HPC):
                nc.vector.tensor_copy(
                    vaug[h][:, 4 * kbg : 4 * kbg + 4, 0:64], tps[h][:]
                )

        # ---- attention + projection, per q-group ----
        ohat = ohat_pool.tile([P, T], F32R, tag="ohat")
        for g in range(NG):
            l_sb = norm_pool.tile([P, G], F32, tag="lsb")
            rinv = norm_pool.tile([P, G], F32, tag="rinv")
            otps_h = [ot_psum.tile([P, G], F32, tag="ot", name=f"ot{h}") for h in range(HPC)]
            n_j = 4 * g + 4
            for jg in range(math.ceil(n_j / 2)):
                js = [j for j in (2 * jg, 2 * jg + 1) if j < n_j]
                diag = 2 * jg >= 4 * g
                stps_h = [mm_psum.tile([P, 2, G], F32, tag="mm", name=f"st{h}") for h in range(HPC)]
                pt_h = [pt_pool.tile([P, 2, G], F32R, tag="pt", name=f"pt{h}") for h in range(HPC)]
                for idx, j in enumerate(js):
                    r = j - 4 * g
                    q0 = 128 * r if r >= 0 else 0
                    for h in range(HPC):
                        hb = 64 * h
                        nc.tensor.matmul(
                            stps_h[h][:, idx, q0:G],
                            kt[hb : hb + 64, ts(j, KB)],
                            qt[hb : hb + 64, G * g + q0 : G * (g + 1)],
                            start=True,
                            stop=True,
                        )
                for h in range(HPC):
                    stps, pt = stps_h[h], pt_h[h]
                    if not diag:
                        nc.scalar.activation(
                            pt[:, :, :],
                            stps[:, :, :],
                            mybir.ActivationFunctionType.Exp,
                            scale=float(HS) ** -0.5,
                        )
                    else:
                        for idx, j in enumerate(js):
                            q0 = 128 * (j - 4 * g)
                            nc.scalar.activation(
                                pt[:, idx, q0:G],
                                stps[:, idx, q0:G],
                                mybir.ActivationFunctionType.Exp,
                                scale=float(HS) ** -0.5,
                            )
                            nc.vector.tensor_tensor(
                                pt[:, idx, q0 : q0 + 128],
                                pt[:, idx, q0 : q0 + 128],
                                tri_sb[:],
                                mybir.AluOpType.mult,
                            )
                for idx, j in enumerate(js):
                    r = j - 4 * g
                    q0 = 128 * r if r >= 0 else 0
                    for h in range(HPC):
                        nc.tensor.matmul(
                            otps_h[h][:, q0:G],
                            vaug[h][:, j, :],
                            pt_h[h][:, idx, q0:G],
                            start=(j == 0),
                            stop=(j == n_j - 1),
                        )
            for h in range(HPC):
                hb = 64 * h
                nc.vector.tensor_copy(l_sb[hb : hb + 64, :], otps_h[h][64:128, :])
            nc.vector.reciprocal_approx_fast(rinv[:], l_sb[:])
            for h in range(HPC):
                hb = 64 * h
                nc.vector.tensor_tensor(
                    ohat[hb : hb + 64, ts(g, G)],
                    otps_h[h][0:64, :],
                    rinv[hb : hb + 64, :],
                    mybir.AluOpType.mult,
                )
            # ---- output projection for this q-group ----
            for tc4 in range(G // P):
                t0 = G * g + P * tc4
                pj = mm_psum.tile([P, C], F32, tag="mm")
                for n in range(C // 512):
                    nc.tensor.matmul(
                        pj[:, ts(n, 512)],
                        ohat[:, t0 : t0 + P],
                        wpt_sb[:, ts(n, 512)],
                        start=True,
                        stop=True,
                    )
                o_sb = out_pool.tile([P, C], F32, tag="osb")
                if tc4 % 2 == 0:
                    nc.vector.tensor_copy(o_sb[:], pj[:])
                else:
                    nc.scalar.copy(o_sb[:], pj[:])
                nc.sync.dma_start(out[b, t0 : t0 + P, :], o_sb[:])
    ctx.close()


def _build():
    if "nc" in _nc_cache:
        return _nc_cache["nc"]
    nc = bacc.Bacc("TRN2", target_bir_lowering=False, debug=False)
    with tile.TileContext(nc) as tc:
        _emit(tc)
    nc.compile()
    _nc_cache["nc"] = nc
    return nc


def kernel(x, wq, wk, wv, w_proj, b_proj):
    x = np.asarray(x, dtype=np.float32)
    wq = np.asarray(wq, dtype=np.float32)
    wk = np.asarray(wk, dtype=np.float32)
    wv = np.asarray(wv, dtype=np.float32)
    w_proj = np.asarray(w_proj, dtype=np.float32)
    b_proj = np.asarray(b_proj, dtype=np.float32)

    nc = _build()

    xt = np.ascontiguousarray(x.transpose(0, 2, 1))  # [B, C, T]
    tri = np.triu(np.ones((P, P), dtype=np.float32))  # keep key <= query
    ident = np.tile(np.eye(64, dtype=np.float32), (2, 1))  # [128, 64]
    ones = np.ones((P, T // KB, 64), dtype=np.float32)

    in_maps = []
    for c in range(NCORES):
        h0 = HPC * c
        in_maps.append(
            {
                "xt": xt,
                "wq2": np.ascontiguousarray(
                    np.concatenate([wq[h0 + i] for i in range(HPC)], axis=1)
                ),
                "wk2": np.ascontiguousarray(
                    np.concatenate([wk[h0 + i] for i in range(HPC)], axis=1)
                ),
                "wv2": np.ascontiguousarray(
                    np.concatenate([wv[h0 + i] for i in range(HPC)], axis=1)
                ),
                "wpt": np.ascontiguousarray(
                    w_proj[:, 128 * c : 128 * (c + 1)].T
                ),
                "tri": tri,
                "ident": ident,
                "ones": ones,
            }
        )

    res = run_bass_kernel_spmd(nc, in_maps, core_ids=list(range(NCORES)))
    acc = np.zeros((B, T, C), dtype=np.float64)
    for r in res.results:
        acc += r["out"]
    return (acc + b_proj).astype(np.float32)


# revision 13
# speedup vs baseline: 1.1954x; 1.1662x over previous
"""Multi-head causal attention (B=2, T=2048, C=1024, H=16, HS=64) on 8 TRN2
NeuronCores.

Sharding: 2 heads per core (tensor parallel). Each core receives the full
(pre-transposed) activations xT [B, C, T], its 2 heads' QKV weight slices
packed [C, 128], and its 128-column slice of w_proj transposed [128, C].
Each core computes a partial output [B, T, C]; the host sums the 8 partials
and adds b_proj.

Per-core kernel (all matmuls in float32r — tf32-like, 1 cycle/row):
  - QT/KT/VT [128(2 heads x 64), T] via lhsT=weight chunks, rhs=xT chunks.
  - V_aug [keys, 128]: V (cols 0:64, via PE-transpose of VT) | ones (64:128).
  - Flash-style causal attention in transposed layout: S^T[keys, q] blocks
    via lhsT=KT block, rhs=QT slice; exp on ScalarE (no max subtraction --
    scores are O(1) by construction); O^T = [V|1].T @ P^T accumulated over
    key blocks gives both O rows (0:64) and the softmax sums l replicated
    (rows 64:128) in one pass.
  - Normalize with reciprocal_approx_fast + mixed-base tensor_tensor.
  - Output projection: lhsT = OhatT t-chunks, rhs = w_projT slice.
"""

import sys

if "/opt/trn_rl_repo" not in sys.path:
    sys.path.insert(0, "/opt/trn_rl_repo")

import math
from contextlib import ExitStack

import numpy as np

import concourse.mybir as mybir
import concourse.tile as tile
from concourse import bacc
from concourse.bass import ts
from concourse.tile_rust import add_dep_helper
from concourse.bass_utils import run_bass_kernel_spmd

B, T, C = 2, 2048, 1024
H, HS = 16, 64
NCORES = 8
HPC = H // NCORES  # 2 heads per core
P = 128
G = 512  # q-group size
NG = T // G  # 4 q-groups
KB = 128  # key block
NPO = C // P  # 8 contraction chunks
F32 = mybir.dt.float32
F32R = mybir.dt.float32r

_nc_cache = {}


def _emit(tc):
    nc = tc.nc
    xt = nc.dram_tensor("xt", [B, C, T], F32R, kind="ExternalInput").ap()
    wq2 = nc.dram_tensor("wq2", [C, 128], F32R, kind="ExternalInput").ap()
    wk2 = nc.dram_tensor("wk2", [C, 128], F32R, kind="ExternalInput").ap()
    wv2 = nc.dram_tensor("wv2", [C, 128], F32R, kind="ExternalInput").ap()
    wpt = nc.dram_tensor("wpt", [128, C], F32R, kind="ExternalInput").ap()
    tri = nc.dram_tensor("tri", [P, P], F32R, kind="ExternalInput").ap()
    out = nc.dram_tensor("out", [B, T, C], F32, kind="ExternalOutput").ap()

    ctx = ExitStack()
    persist = ctx.enter_context(tc.tile_pool(name="persist", bufs=1))
    xt_pool = ctx.enter_context(tc.tile_pool(name="xtp", bufs=1))
    qk_pool = ctx.enter_context(tc.tile_pool(name="qkp", bufs=2))
    vt_pool = ctx.enter_context(tc.tile_pool(name="vtp", bufs=1))
    vaug_pool = ctx.enter_context(tc.tile_pool(name="vaugp", bufs=2))
    pt_pool = ctx.enter_context(tc.tile_pool(name="ptp", bufs=4))
    norm_pool = ctx.enter_context(tc.tile_pool(name="normp", bufs=2))
    ohat_pool = ctx.enter_context(tc.tile_pool(name="ohatp", bufs=2))
    out_pool = ctx.enter_context(tc.tile_pool(name="outp", bufs=2))
    st_psum = ctx.enter_context(tc.tile_pool(name="stps", bufs=2, space="PSUM"))
    ot_psum = ctx.enter_context(tc.tile_pool(name="otps", bufs=2, space="PSUM"))
    mm_psum = ctx.enter_context(tc.tile_pool(name="mmps", bufs=2, space="PSUM"))

    # persistent weights / masks
    wq_sb = persist.tile([P, NPO, 128], F32R, tag="wq")
    wk_sb = persist.tile([P, NPO, 128], F32R, tag="wk")
    wv_sb = persist.tile([P, NPO, 128], F32R, tag="wv")
    nc.sync.dma_start(wq_sb[:], wq2.rearrange("(po pi) d -> pi po d", pi=P))
    nc.sync.dma_start(wk_sb[:], wk2.rearrange("(po pi) d -> pi po d", pi=P))
    nc.sync.dma_start(wv_sb[:], wv2.rearrange("(po pi) d -> pi po d", pi=P))
    wpt_sb = persist.tile([P, C], F32R, tag="wpt")
    nc.sync.dma_start(wpt_sb[:], wpt[:])
    tri_sb = persist.tile([P, P], F32R, tag="tri")
    nc.sync.dma_start(tri_sb[:], tri[:])
    # [128, 64] identity (both partition halves = I64) for VT transposes
    identd = nc.dram_tensor("ident", [P, 64], F32R, kind="ExternalInput").ap()
    ident = persist.tile([P, 64], F32R, tag="ident")
    nc.sync.dma_start(ident[:], identd[:])
    onesd = nc.dram_tensor("ones", [P, T // KB, 64], F32R, kind="ExternalInput").ap()

    for b in range(B):
        # ---- load xT[b] ----
        xt_sb = xt_pool.tile([P, NPO, T], F32R, tag="xt")
        xt_dmas = []
        for po in range(NPO):
            i = nc.sync.dma_start(xt_sb[:, po, :], xt[b, po * P : (po + 1) * P, :])
            if po >= 2:
                add_dep_helper(i.ins, xt_dmas[po - 2].ins, sync=True)
            xt_dmas.append(i)

        # ---- QT / KT / VT projections: [128(2h x 64), T] ----
        qt = qk_pool.tile([P, T], F32R, tag="qt")
        kt = qk_pool.tile([P, T], F32R, tag="kt")
        vt = vt_pool.tile([P, T], F32R, tag="vt")
        for w_sb, dst in ((wq_sb, qt), (wk_sb, kt), (wv_sb, vt)):
            for tg in range(T // 512):
                ps = mm_psum.tile([P, 512], F32, tag="mm")
                for po in range(NPO):
                    nc.tensor.matmul(
                        ps[:],
                        w_sb[:, po, :],
                        xt_sb[:, po, ts(tg, 512)],
                        start=(po == 0),
                        stop=(po == NPO - 1),
                    )
                nc.vector.tensor_copy(dst[:, ts(tg, 512)], ps[:])

        # ---- V_aug per head: [keys(P), 16 kb, 128] = [V | ones] ----
        vaug = []
        for h in range(HPC):
            va = vaug_pool.tile([P, T // KB, 128], F32R, tag=f"vaug{h}")
            nc.sync.dma_start(va[:, :, 64:128], onesd[:])
            vaug.append(va)
        for kbg in range(4):
            tps = [mm_psum.tile([P, 4, 64], F32R, tag="mm", name=f"vtr{h}") for h in range(HPC)]
            for kk in range(4):
                kb = 4 * kbg + kk
                for h in range(HPC):
                    nc.tensor.transpose(
                        tps[h][:, kk, :],
                        vt[64 * h : 64 * h + 64, ts(kb, KB)],
                        ident[64 * h : 64 * h + 64, :],
                    )
            for h in range(HPC):
                nc.vector.tensor_copy(
                    vaug[h][:, 4 * kbg : 4 * kbg + 4, 0:64], tps[h][:]
                )

        # ---- attention + projection, per q-group ----
        ohat = ohat_pool.tile([P, T], F32R, tag="ohat")
        for g in range(NG):
            l_sb = norm_pool.tile([P, G], F32, tag="lsb")
            rinv = norm_pool.tile([P, G], F32, tag="rinv")
            otps_h = [ot_psum.tile([P, G], F32, tag="ot", name=f"ot{h}") for h in range(HPC)]
            n_j = 4 * g + 4
            for jg in range(math.ceil(n_j / 2)):
                js = [j for j in (2 * jg, 2 * jg + 1) if j < n_j]
                diag = 2 * jg >= 4 * g
                stps_h = [st_psum.tile([P, 2, G], F32, tag="st", name=f"st{h}") for h in range(HPC)]
                pt_h = [pt_pool.tile([P, 2, G], F32R, tag="pt", name=f"pt{h}") for h in range(HPC)]
                for idx, j in enumerate(js):
                    r = j - 4 * g
                    q0 = 128 * r if r >= 0 else 0
                    for h in range(HPC):
                        hb = 64 * h
                        nc.tensor.matmul(
                            stps_h[h][:, idx, q0:G],
                            kt[hb : hb + 64, ts(j, KB)],
                            qt[hb : hb + 64, G * g + q0 : G * (g + 1)],
                            start=True,
                            stop=True,
                        )
                for h in range(HPC):
                    stps, pt = stps_h[h], pt_h[h]
                    if not diag:
                        nc.scalar.activation(
                            pt[:, :, :],
                            stps[:, :, :],
                            mybir.ActivationFunctionType.Exp,
                            scale=float(HS) ** -0.5,
                        )
                    else:
                        for idx, j in enumerate(js):
                            q0 = 128 * (j - 4 * g)
                            nc.scalar.activation(
                                pt[:, idx, q0:G],
                                stps[:, idx, q0:G],
                                mybir.ActivationFunctionType.Exp,
                                scale=float(HS) ** -0.5,
                            )
                            nc.vector.tensor_tensor(
                                pt[:, idx, q0 : q0 + 128],
                                pt[:, idx, q0 : q0 + 128],
                                tri_sb[:],
                                mybir.AluOpType.mult,
                            )
                for idx, j in enumerate(js):
                    r = j - 4 * g
                    q0 = 128 * r if r >= 0 else 0
                    for h in range(HPC):
                        nc.tensor.matmul(
                            otps_h[h][:, q0:G],
                            vaug[h][:, j, :],
                            pt_h[h][:, idx, q0:G],
                            start=(j == 0),
                            stop=(j == n_j - 1),
                        )
            for h in range(HPC):
                hb = 64 * h
                nc.vector.tensor_copy(l_sb[hb : hb + 64, :], otps_h[h][64:128, :])
            nc.vector.reciprocal_approx_fast(rinv[:], l_sb[:])
            for h in range(HPC):
                hb = 64 * h
                nc.vector.tensor_tensor(
                    ohat[hb : hb + 64, ts(g, G)],
                    otps_h[h][0:64, :],
                    rinv[hb : hb + 64, :],
                    mybir.AluOpType.mult,
                )
            # ---- output projection (delayed by one q-group) ----
            def emit_proj(g, b=b, ohat=ohat):
              for tc4 in range(G // P):
                t0 = G * g + P * tc4
                o_sb = out_pool.tile([P, C], F32, tag="osb")
                for n in range(C // 512):
                    pj = mm_psum.tile([P, 512], F32, tag="mm", name=f"pj{n}")
                    nc.tensor.matmul(
                        pj[:],
                        ohat[:, t0 : t0 + P],
                        wpt_sb[:, ts(n, 512)],
                        start=True,
                        stop=True,
                    )
                    if (2 * tc4 + n) % 2 == 0:
                        nc.vector.tensor_copy(o_sb[:, ts(n, 512)], pj[:])
                    else:
                        nc.scalar.copy(o_sb[:, ts(n, 512)], pj[:])
                nc.sync.dma_start(out[b, t0 : t0 + P, :], o_sb[:])
            if g > 0:
                emit_proj(g - 1)
            if g == NG - 1:
                emit_proj(g)
    ctx.close()


def _build():
    if "nc" in _nc_cache:
        return _nc_cache["nc"]
    nc = bacc.Bacc("TRN2", target_bir_lowering=False, debug=False)
    with tile.TileContext(nc) as tc:
        _emit(tc)
    nc.compile()
    _nc_cache["nc"] = nc
    return nc


def kernel(x, wq, wk, wv, w_proj, b_proj):
    x = np.asarray(x, dtype=np.float32)
    wq = np.asarray(wq, dtype=np.float32)
    wk = np.asarray(wk, dtype=np.float32)
    wv = np.asarray(wv, dtype=np.float32)
    w_proj = np.asarray(w_proj, dtype=np.float32)
    b_proj = np.asarray(b_proj, dtype=np.float32)

    nc = _build()

    xt = np.ascontiguousarray(x.transpose(0, 2, 1))  # [B, C, T]
    tri = np.triu(np.ones((P, P), dtype=np.float32))  # keep key <= query
    ident = np.tile(np.eye(64, dtype=np.float32), (2, 1))  # [128, 64]
    ones = np.ones((P, T // KB, 64), dtype=np.float32)

    in_maps = []
    for c in range(NCORES):
        h0 = HPC * c
        in_maps.append(
            {
                "xt": xt,
                "wq2": np.ascontiguousarray(
                    np.concatenate([wq[h0 + i] for i in range(HPC)], axis=1)
                ),
                "wk2": np.ascontiguousarray(
                    np.concatenate([wk[h0 + i] for i in range(HPC)], axis=1)
                ),
                "wv2": np.ascontiguousarray(
                    np.concatenate([wv[h0 + i] for i in range(HPC)], axis=1)
                ),
                "wpt": np.ascontiguousarray(
                    w_proj[:, 128 * c : 128 * (c + 1)].T
                ),
                "tri": tri,
                "ident": ident,
                "ones": ones,
            }
        )

    res = run_bass_kernel_spmd(nc, in_maps, core_ids=list(range(NCORES)))
    acc = np.zeros((B, T, C), dtype=np.float64)
    for r in res.results:
        acc += r["out"]
    return (acc + b_proj).astype(np.float32)
